# revision 1
# baseline (speedup 1.0000x reference)
"""3-layer GAT (graph attention network) forward pass on 8 Trainium2 cores.

v3 — slot-per-partition edge layout + hardware dma_gather:

  - nodes are binned per core into 98 blocks of 128 sorted by in-degree;
    each dst node owns ONE SBUF partition of its block, so er is a free
    per-partition broadcast (no er gather, no one-hot build) and the
    segment-sum is a PSUM accumulation of edge-tiles against a constant
    identity (one bf16 matmul per tile).
  - per-edge source rows are fetched with the Q7 `dma_gather` HW gather
    (flat int16 indices, 256B-multiple rows).  int16 limits one gather to
    32K table rows, so the AllGathered table is split into 4 chunks of
    2 cores (25088 rows) and each block issues up to 4 gathers; a block's
    edge-tiles are grouped by source chunk, padded per (block, chunk) to
    the cross-core max count (measured ~2.5x tile inflation, the price of
    int16 indices — still ~7x cheaper than per-tile SWDGE indirect DMAs
    at ~1.1us fixed cost each).
  - table rows are 256B (layers 0/1: [Wh bf16 x64 | el f32 x4 | pad]) or
    512B (layer 2: [W2h bf16 x160 | el2 f32 x4 | pad]); padded edge slots
    point at a pad row whose el is patched to -1e38 so exp() gives 0.
  - bf16 everywhere on the PE; f32 attention logits and accumulators.
"""

import numpy as np
import ml_dtypes

import concourse.bass as bass
import concourse.bacc as bacc
import concourse.tile as tile
from concourse import mybir, bass_utils, library_config

F32 = mybir.dt.float32
I16 = mybir.dt.int16
BF16 = mybir.dt.bfloat16

BF_NP = ml_dtypes.bfloat16

NCHUNK = 4          # int16 gather chunks (2 cores each)
PADLOC = None       # set from cfg: NOWN - 1 (chunk-local pad row id)


class Cfg:
    def __init__(self, n, e, ncores, in_dim=128, hid=16, heads=4, outc=40,
                 neg=0.2, eps=1e-5):
        assert n % ncores == 0
        self.N, self.E, self.NCORES = n, e, ncores
        self.IN, self.HID, self.HEADS, self.OUTC = in_dim, hid, heads, outc
        self.F = heads * hid          # 64
        self.F2 = heads * outc        # 160
        self.P = 128
        self.NPC = n // ncores        # real nodes per core
        self.BLOCKS = (self.NPC + self.P - 1) // self.P
        self.NOWN = self.BLOCKS * self.P   # padded nodes per core
        self.NPAD = ncores * self.NOWN
        self.NEG, self.EPS = neg, eps


CFG = Cfg(100000, 1600000, 8)


# ---------------------------------------------------------------- host prep

def preprocess(cfg, src, dst):
    """Degree-sorted binning + slot-per-partition chunked edge layout.

    Returns (idx16 [NC, 128, 8*TOT] int16, Tbc [B, 4], boff [B, 4],
    Tp [B], goff [B+1], newlocal).
    Edge-tile tau = goff[b] + boff[b, c] + rank lands at SBUF column tau of
    its block's gather output; flat gather position = tau*128 + slot.
    """
    src = np.asarray(src, np.int64)
    dst = np.asarray(dst, np.int64)
    core_of = np.arange(cfg.N) // cfg.NPC
    # chunk of an edge = 2-core group of its source (cores don't move)
    chunk_node = core_of // 2
    # per-dst per-chunk in-degree vectors; lex-sort nodes per core so blocks
    # group nodes with similar vectors (cross-core order statistics align),
    # cutting the per-(block, chunk) cross-core max padding.
    cd = np.zeros((cfg.N, NCHUNK), np.int64)
    np.add.at(cd, (dst, chunk_node[src]), 1)
    newlocal = np.empty(cfg.N, np.int64)
    for r in range(cfg.NCORES):
        lo, hi = r * cfg.NPC, (r + 1) * cfg.NPC
        v = cd[lo:hi]
        order = np.lexsort((-v[:, 3], -v[:, 2], -v[:, 1], -v[:, 0]))
        newlocal[lo + order] = np.arange(cfg.NPC)

    newglobal = core_of * cfg.NOWN + newlocal
    chunk_rows = 2 * cfg.NOWN
    chunk_of = newglobal // chunk_rows            # [N] 0..3
    local16 = (newglobal - chunk_of * chunk_rows).astype(np.int16)

    dl = newlocal[dst]
    r_e = core_of[dst]
    b_e = dl // cfg.P
    p_e = dl % cfg.P
    c_e = chunk_of[src]

    # per (r, b, c, p) counts -> cross-core per (b, c) tile counts
    key = ((r_e * cfg.BLOCKS + b_e) * NCHUNK + c_e) * cfg.P + p_e
    cnt = np.bincount(key, minlength=cfg.NCORES * cfg.BLOCKS * NCHUNK * cfg.P)
    cnt4 = cnt.reshape(cfg.NCORES, cfg.BLOCKS, NCHUNK, cfg.P)
    Tbc = cnt4.max(axis=(0, 3)).astype(np.int64)          # [B, 4]
    boff = np.concatenate(
        [np.zeros((cfg.BLOCKS, 1), np.int64), np.cumsum(Tbc, axis=1)], axis=1)
    Tp = boff[:, NCHUNK]                                  # [B]
    goff = np.concatenate([[0], np.cumsum(Tp)]).astype(np.int64)
    TOT = int(goff[-1])

    order_e = np.argsort(key, kind="stable")
    gstart = np.concatenate([[0], np.cumsum(cnt)])
    rank = np.arange(cfg.E) - gstart[key[order_e]]

    bs = b_e[order_e]
    cs = c_e[order_e]
    tau = goff[bs] + boff[bs, cs] + rank
    flat = tau * cfg.P + p_e[order_e]
    rs = r_e[order_e]
    vals = local16[src][order_e]

    idx16 = np.full((cfg.NCORES, cfg.P * TOT), cfg.NOWN - 1, np.int16)
    idx16[rs, flat] = vals
    # wrap: flat position i lives at [i % 16, i // 16], replicated x8 rows
    wrapped = idx16.reshape(cfg.NCORES, cfg.P * TOT // 16, 16)
    wrapped = np.ascontiguousarray(wrapped.transpose(0, 2, 1))  # [NC,16,S]
    idx16 = np.ascontiguousarray(
        np.tile(wrapped, (1, 8, 1)))                      # [NC, 128, S]
    return idx16, Tbc.astype(int), boff.astype(int), Tp.astype(int), goff, newlocal


def _fold_el(W, a, heads, dph):
    return np.einsum("khd,hd->kh", W.reshape(W.shape[0], heads, dph),
                     a).astype(np.float32)


# ---------------------------------------------------------------- program

def build_layer_program(cfg, Tbc, boff, Tp, goff, layer):
    P, F, F2, IN = cfg.P, cfg.F, cfg.F2, cfg.IN
    H, D, OC = cfg.HEADS, cfg.HID, cfg.OUTC
    B = cfg.BLOCKS
    fo = F2 if layer == 2 else F
    fin = IN if layer == 0 else F
    ELEM = 256 if layer == 2 else 128   # bf16 units per table row (256B/512B)
    # V rows carry only Wh*ee; denominators reduce from ee directly
    Tpmax = int(max(Tp))
    TOT = int(goff[-1])
    GA = 7                              # phase-A write group (98 = 14*7)
    GI = 7                              # idx-load group
    assert B % GA == 0 and B % GI == 0
    islab_w = int(max(8 * (goff[g + GI] - goff[g])
                      for g in range(0, B, GI)))
    chunk_rows = 2 * cfg.NOWN

    nc = bacc.Bacc("TRN2", target_bir_lowering=False, debug=False,
                   num_devices=cfg.NCORES, num_swdge_queues=4)

    def inp(name, shape, dt=F32):
        return nc.dram_tensor(name, shape, dt, kind="ExternalInput").ap()

    x_in = inp("x_in", [fin, cfg.NOWN], BF16)
    idx_in = inp("idx16", [P, 8 * TOT], I16)
    Wext_i = inp("Wext", [fin, fo + 8], BF16)
    ident_i = inp("ident", [P, P], F32)
    if layer < 2:
        gamma_i = inp("gamma", [F, 1])
        beta_i = inp("beta", [F, 1])
        y_out = nc.dram_tensor("y_out", [F, cfg.NOWN], BF16,
                               kind="ExternalOutput").ap()
    if layer == 0:
        resW_i = inp("resW", [IN, F], BF16)
    if layer == 2:
        resW_i = inp("resW", [F, F2], BF16)
        b2bc_in = inp("b2bc", [P, F2])
        out = nc.dram_tensor("out", [cfg.NOWN, F2], F32,
                             kind="ExternalOutput").ap()

    rg = [list(range(cfg.NCORES))]

    with tile.TileContext(nc) as tc:
        with (
            tc.tile_pool(name="big", bufs=1) as bigp,
            tc.tile_pool(name="const", bufs=1) as cons,
            tc.tile_pool(name="work", bufs=2) as wk,
            tc.tile_pool(name="idxp", bufs=2) as idxp,
            tc.tile_pool(name="grp", bufs=2) as grp,
            tc.tile_pool(name="ps", bufs=2, space="PSUM") as ps,
            tc.tile_pool(name="dram", bufs=1, space="DRAM") as dr,
        ):
            nc.gpsimd.load_library(library_config.mlp)

            tbl_own = dr.tile([cfg.NOWN, ELEM], BF16)
            tbl_full = dr.tile([cfg.NPAD, ELEM], BF16, addr_space="Shared")
            if layer < 2:
                stats_in = dr.tile([F, 2], F32)
                stats_out = dr.tile([F, 2], F32, addr_space="Shared")

            xT = bigp.tile([fin, cfg.NOWN], BF16, tag="xT")
            nc.sync.dma_start(out=xT[:], in_=x_in[:])
            ersb = bigp.tile([P, B * H], F32, tag="ersb")
            if layer < 2:
                yT = bigp.tile([F, cfg.NOWN], BF16, tag="yT")

            Wext = cons.tile([fin, fo + 8], BF16, tag="Wext")
            nc.sync.dma_start(out=Wext[:], in_=Wext_i[:])
            idf = cons.tile([P, P], F32, tag="idf")
            nc.sync.dma_start(out=idf[:], in_=ident_i[:])
            if layer < 2:
                gam = cons.tile([F, 1], F32, tag="gam")
                bet = cons.tile([F, 1], F32, tag="bet")
                nc.sync.dma_start(out=gam[:], in_=gamma_i[:])
                nc.sync.dma_start(out=bet[:], in_=beta_i[:])
                stats = cons.tile([F, 2], F32, tag="stats")
                nc.vector.memset(stats[:], 0.0)
            if layer == 0:
                rW = cons.tile([IN, F], BF16, tag="rW")
                nc.sync.dma_start(out=rW[:], in_=resW_i[:])
            if layer == 2:
                rW = cons.tile([F, F2], BF16, tag="rW")
                nc.sync.dma_start(out=rW[:], in_=resW_i[:])
                b2bc = cons.tile([P, F2], F32, tag="b2bc")
                nc.sync.dma_start(out=b2bc[:], in_=b2bc_in[:])

            # ---- phase A: table rows [Wh | el | pad] + er, AllGather ----
            for g in range(B // GA):
                grow = grp.tile([P, GA * ELEM], BF16, tag="grow")
                for k in range(GA):
                    b = g * GA + k
                    tp = ps.tile([P, fo + 8], F32, tag="tp")
                    nc.tensor.matmul(out=tp[:], lhsT=xT[:, b * P:(b + 1) * P],
                                     rhs=Wext[:], start=True, stop=True)
                    seg = grow[:, k * ELEM:k * ELEM + fo + 8]
                    nc.vector.tensor_copy(out=seg[:, :fo], in_=tp[:, :fo])
                    nc.vector.tensor_copy(out=seg[:, fo:fo + 8].bitcast(F32),
                                          in_=tp[:, fo:fo + 4])
                    nc.vector.tensor_copy(out=ersb[:, b * H:(b + 1) * H],
                                          in_=tp[:, fo + 4:fo + 8])
                if g == B // GA - 1:
                    # pad-row patch: el of rows NOWN-32..NOWN (all zero pad
                    # nodes; every chunk's PADLOC row is one of these on an
                    # even core) = -1e38 so padded edge slots get ee = 0.
                    nc.vector.memset(
                        grow[P - 32:P,
                             (GA - 1) * ELEM + fo:(GA - 1) * ELEM + fo + 8]
                        .bitcast(F32), -1e38)
                nc.sync.dma_start(
                    out=tbl_own[g * GA * P:(g + 1) * GA * P, :]
                    .rearrange("(k p) f -> p k f", p=P),
                    in_=grow[:].rearrange("p (k f) -> p k f", k=GA))
            nc.gpsimd.collective_compute(
                "AllGather", mybir.AluOpType.bypass, replica_groups=rg,
                ins=[tbl_own[:].opt()], outs=[tbl_full[:].opt()])
            # gathers race the collective's remote writes without this:
            tc.strict_bb_all_engine_barrier()

            # ---- phase B: edge aggregation per block ----
            qrot = [0]
            for b in range(B):
                T = int(Tp[b])
                if b % GI == 0:
                    # idx slab for blocks [b, b+GI): wrapped columns
                    i0, i1 = 8 * int(goff[b]), 8 * int(goff[b + GI])
                    islab = idxp.tile([P, islab_w], I16, tag="islab")
                    nc.sync.dma_start(out=islab[:, :i1 - i0],
                                      in_=idx_in[:, i0:i1])
                    islab0 = i0
                G = wk.tile([P, Tpmax * ELEM], BF16, tag="G")
                for c in range(NCHUNK):
                    Tc = int(Tbc[b, c])
                    bo = int(boff[b, c])
                    for s in range(0, Tc, 8):
                        w = min(8, Tc - s)
                        f0 = 8 * (int(goff[b]) + bo + s) - islab0
                        nc.gpsimd.dma_gather(
                            G[:, (bo + s) * ELEM:(bo + s + w) * ELEM]
                            .rearrange("p (j e) -> p j e", e=ELEM),
                            tbl_full[c * chunk_rows:(c + 1) * chunk_rows, :],
                            islab[:, f0:f0 + 8 * w],
                            128 * w, 128 * w, ELEM,
                            queue_num=qrot[0] % 4, single_packet=False)
                        qrot[0] += 1

                G3 = G[:, :T * ELEM].rearrange("p (t f) -> p t f", t=T)
                el = G3[:, :, fo:fo + 8].bitcast(F32)      # [P, T, H]

                ee = wk.tile([P, Tpmax * H], F32, tag="ee")
                ee3 = ee[:, :T * H].rearrange("p (t h) -> p t h", t=T)
                nc.vector.tensor_tensor(
                    out=ee3, in0=el,
                    in1=ersb[:, b * H:(b + 1) * H].unsqueeze(1)
                    .to_broadcast([P, T, H]),
                    op=mybir.AluOpType.add)
                esc = wk.tile([P, Tpmax * H], F32, tag="esc")
                nc.vector.tensor_scalar_mul(out=esc[:, :T * H],
                                            in0=ee[:, :T * H],
                                            scalar1=cfg.NEG)
                nc.vector.tensor_tensor(out=ee[:, :T * H], in0=ee[:, :T * H],
                                        in1=esc[:, :T * H],
                                        op=mybir.AluOpType.max)
                nc.scalar.activation(out=ee[:, :T * H], in_=ee[:, :T * H],
                                     func=mybir.ActivationFunctionType.Exp)

                V = wk.tile([P, Tpmax * fo], BF16, tag="V")
                V3 = V[:, :T * fo].rearrange("p (t f) -> p t f", t=T)
                nc.vector.tensor_tensor(
                    out=V3.rearrange("p t (h d) -> p t h d", h=H),
                    in0=G3[:, :, :fo].rearrange("p t (h d) -> p t h d", h=H),
                    in1=ee3.unsqueeze(3).to_broadcast(
                        [P, T, H, OC if layer == 2 else D]),
                    op=mybir.AluOpType.mult)

                if layer < 2:
                    # node-major aggregation: one strided DVE reduce over
                    # the tile axis (pads contribute exactly 0), then
                    # normalize and transpose to feature-major on the PE.
                    acc = wk.tile([P, fo], F32, tag="acc")
                    nc.vector.reduce_sum(
                        out=acc[:],
                        in_=V[:, :T * fo].rearrange("p (t f) -> p f t", t=T),
                        axis=mybir.AxisListType.X)
                    den = wk.tile([P, H], F32, tag="den")
                    nc.vector.reduce_sum(
                        out=den[:],
                        in_=ee[:, :T * H].rearrange("p (t h) -> p h t", t=T),
                        axis=mybir.AxisListType.X)
                    dmax = wk.tile([P, H], F32, tag="dmax")
                    nc.vector.tensor_scalar_max(out=dmax[:], in0=den[:],
                                                scalar1=1e-16)
                    rec = wk.tile([P, H], F32, tag="rec")
                    nc.vector.reciprocal(out=rec[:], in_=dmax[:])
                    rstn = wk.tile([P, fo], F32, tag="rstn")
                    nc.vector.tensor_tensor(
                        out=rstn[:].rearrange("p (h d) -> p h d", h=H),
                        in0=acc[:].rearrange("p (h d) -> p h d", h=H),
                        in1=rec[:].unsqueeze(2).to_broadcast([P, H, D]),
                        op=mybir.AluOpType.mult)
                    trp = ps.tile([fo, P], F32, tag="trp")
                    nc.tensor.transpose(out=trp[:], in_=rstn[:],
                                        identity=idf[:])

                    hsl = yT[:, b * P:(b + 1) * P]
                    if layer == 0:
                        rps = ps.tile([F, P], F32, tag="res")
                        nc.tensor.matmul(out=rps[:], lhsT=rW[:],
                                         rhs=xT[:, b * P:(b + 1) * P],
                                         start=True, stop=True)
                        nc.vector.tensor_copy(out=hsl, in_=trp[:])
                        nc.vector.tensor_tensor(out=hsl, in0=hsl, in1=rps[:],
                                                op=mybir.AluOpType.add)
                    else:
                        hold = xT[:, b * P:(b + 1) * P]
                        nc.vector.tensor_tensor(out=hsl, in0=trp[:],
                                                in1=hold,
                                                op=mybir.AluOpType.add)
                        nc.vector.tensor_tensor(out=hsl, in0=hsl, in1=hold,
                                                op=mybir.AluOpType.add)

                    red = wk.tile([F, 1], F32, tag="red")
                    nc.vector.reduce_sum(out=red[:], in_=hsl,
                                         axis=mybir.AxisListType.X)
                    nc.vector.tensor_tensor(out=stats[:, 0:1],
                                            in0=stats[:, 0:1], in1=red[:],
                                            op=mybir.AluOpType.add)
                    sq = wk.tile([F, P], F32, tag="sq")
                    red2 = wk.tile([F, 1], F32, tag="red2")
                    nc.scalar.activation(
                        out=sq[:], in_=hsl,
                        func=mybir.ActivationFunctionType.Square,
                        accum_out=red2[:])
                    nc.vector.tensor_tensor(out=stats[:, 1:2],
                                            in0=stats[:, 1:2], in1=red2[:],
                                            op=mybir.AluOpType.add)
                else:
                    # node-major aggregation via strided DVE reduces
                    acc = wk.tile([P, fo], F32, tag="acc")
                    nc.vector.reduce_sum(
                        out=acc[:],
                        in_=V[:, :T * fo].rearrange("p (t f) -> p f t", t=T),
                        axis=mybir.AxisListType.X)
                    den = wk.tile([P, H], F32, tag="den")
                    nc.vector.reduce_sum(
                        out=den[:],
                        in_=ee[:, :T * H].rearrange("p (t h) -> p h t", t=T),
                        axis=mybir.AxisListType.X)
                    dmax = wk.tile([P, H], F32, tag="dmax")
                    nc.vector.tensor_scalar_max(out=dmax[:], in0=den[:],
                                                scalar1=1e-16)
                    rec = wk.tile([P, H], F32, tag="rec")
                    nc.vector.reciprocal(out=rec[:], in_=dmax[:])
                    rps = ps.tile([P, F2], F32, tag="res")
                    nc.tensor.matmul(out=rps[:],
                                     lhsT=xT[:, b * P:(b + 1) * P],
                                     rhs=rW[:], start=True, stop=True)
                    rst = wk.tile([P, F2], F32, tag="rst")
                    nc.vector.tensor_tensor(
                        out=rst[:].rearrange("p (h o) -> p h o", h=H),
                        in0=acc[:].rearrange("p (h o) -> p h o", h=H),
                        in1=rec[:].unsqueeze(2).to_broadcast([P, H, OC]),
                        op=mybir.AluOpType.mult)
                    nc.vector.tensor_tensor(out=rst[:], in0=rst[:],
                                            in1=rps[:],
                                            op=mybir.AluOpType.add)
                    nc.vector.tensor_tensor(out=rst[:], in0=rst[:],
                                            in1=b2bc[:],
                                            op=mybir.AluOpType.add)
                    nc.sync.dma_start(out=out[b * P:(b + 1) * P, :],
                                      in_=rst[:])

            # ---- phase C: BN (layers 0/1) + writeback ----
            if layer < 2:
                nc.sync.dma_start(out=stats_in[:], in_=stats[:])
                nc.gpsimd.collective_compute(
                    "AllReduce", mybir.AluOpType.add, replica_groups=rg,
                    ins=[stats_in[:].opt()], outs=[stats_out[:].opt()])
                tc.strict_bb_all_engine_barrier()
                ssb = cons.tile([F, 2], F32, tag="ssb")
                nc.sync.dma_start(out=ssb[:], in_=stats_out[:])
                mu = cons.tile([F, 1], F32, tag="mu")
                tmp1 = cons.tile([F, 1], F32, tag="tmp1")
                scl = cons.tile([F, 1], F32, tag="scl")
                bia = cons.tile([F, 1], F32, tag="bia")
                musq = cons.tile([F, 1], F32, tag="musq")
                invn = 1.0 / cfg.N
                nc.vector.tensor_scalar_mul(out=mu[:], in0=ssb[:, 0:1],
                                            scalar1=invn)
                nc.vector.tensor_scalar_mul(out=tmp1[:], in0=ssb[:, 1:2],
                                            scalar1=invn)
                nc.scalar.activation(out=musq[:], in_=mu[:],
                                     func=mybir.ActivationFunctionType.Square)
                nc.vector.tensor_tensor(out=tmp1[:], in0=tmp1[:],
                                        in1=musq[:],
                                        op=mybir.AluOpType.subtract)
                nc.vector.tensor_scalar_add(out=tmp1[:], in0=tmp1[:],
                                            scalar1=cfg.EPS)
                nc.scalar.activation(out=tmp1[:], in_=tmp1[:],
                                     func=mybir.ActivationFunctionType.Sqrt)
                nc.vector.reciprocal(out=tmp1[:], in_=tmp1[:])
                nc.vector.tensor_tensor(out=scl[:], in0=tmp1[:], in1=gam[:],
                                        op=mybir.AluOpType.mult)
                nc.vector.tensor_tensor(out=tmp1[:], in0=mu[:], in1=scl[:],
                                        op=mybir.AluOpType.mult)
                nc.vector.tensor_tensor(out=bia[:], in0=bet[:], in1=tmp1[:],
                                        op=mybir.AluOpType.subtract)
                nc.scalar.activation(out=yT[:, :], in_=yT[:, :],
                                     func=mybir.ActivationFunctionType.Relu,
                                     scale=scl[:, 0:1], bias=bia[:, 0:1])
                if cfg.NPC < cfg.NOWN:
                    nc.vector.memset(yT[:, cfg.NPC:cfg.NOWN], 0.0)
                nc.sync.dma_start(out=y_out[:], in_=yT[:])

    nc.compile()
    return nc


# ---------------------------------------------------------------- host glue

def make_in_maps(cfg, inputs, idx16, newlocal):
    feat = np.asarray(inputs["feat"], np.float32)
    H, D, OC = cfg.HEADS, cfg.HID, cfg.OUTC
    W0 = np.asarray(inputs["W0"], np.float32)
    W1 = np.asarray(inputs["W1"], np.float32)
    W2 = np.asarray(inputs["W2"], np.float32)
    W0ext = np.concatenate(
        [W0, _fold_el(W0, np.asarray(inputs["al0"]), H, D),
         _fold_el(W0, np.asarray(inputs["ar0"]), H, D)], axis=1).astype(BF_NP)
    W1ext = np.concatenate(
        [W1, _fold_el(W1, np.asarray(inputs["al1"]), H, D),
         _fold_el(W1, np.asarray(inputs["ar1"]), H, D)], axis=1).astype(BF_NP)
    W2ext = np.concatenate(
        [W2, _fold_el(W2, np.asarray(inputs["al2"]), H, OC),
         _fold_el(W2, np.asarray(inputs["ar2"]), H, OC)], axis=1).astype(BF_NP)

    ident = np.eye(cfg.P, dtype=np.float32)

    layer_maps = [[], [], []]
    for r in range(cfg.NCORES):
        ids = np.arange(r * cfg.NPC, (r + 1) * cfg.NPC)
        fp = np.zeros((cfg.NOWN, cfg.IN), np.float32)
        fp[newlocal[ids]] = feat[ids]
        common = {"idx16": idx16[r], "ident": ident}
        layer_maps[0].append({
            "x_in": np.ascontiguousarray(fp.T).astype(BF_NP),
            "Wext": W0ext,
            "resW": np.asarray(inputs["resW0"], np.float32).astype(BF_NP),
            "gamma": np.asarray(inputs["gamma0"], np.float32).reshape(cfg.F, 1),
            "beta": np.asarray(inputs["beta0"], np.float32).reshape(cfg.F, 1),
            **common,
        })
        layer_maps[1].append({
            "Wext": W1ext,
            "gamma": np.asarray(inputs["gamma1"], np.float32).reshape(cfg.F, 1),
            "beta": np.asarray(inputs["beta1"], np.float32).reshape(cfg.F, 1),
            **common,
        })
        layer_maps[2].append({
            "Wext": W2ext,
            "resW": np.asarray(inputs["resW2"], np.float32).astype(BF_NP),
            "b2bc": np.tile(np.asarray(inputs["b2"], np.float32)
                            .reshape(1, cfg.F2), (cfg.P, 1)),
            **common,
        })
    return layer_maps


def assemble_output(cfg, results, newlocal):
    out = np.empty((cfg.N, cfg.F2), np.float32)
    for r in range(cfg.NCORES):
        ids = np.arange(r * cfg.NPC, (r + 1) * cfg.NPC)
        out[ids] = results[r]["out"][newlocal[ids]]
    return out


_PROG_CACHE = {}


def get_program(cfg, Tbc, boff, Tp, goff, layer):
    key = (cfg.N, cfg.E, int(goff[-1]), tuple(Tp), layer)
    if key not in _PROG_CACHE:
        _PROG_CACHE[key] = build_layer_program(cfg, Tbc, boff, Tp, goff, layer)
    return _PROG_CACHE[key]


def run(inputs, trace=False, trace_cores=None):
    cfg = CFG
    src = np.asarray(inputs["src"])
    dst = np.asarray(inputs["dst"])
    idx16, Tbc, boff, Tp, goff, newlocal = preprocess(cfg, src, dst)
    layer_maps = make_in_maps(cfg, inputs, idx16, newlocal)
    cores = list(range(cfg.NCORES))
    total_ns = 0
    layer_res = []
    for layer in range(3):
        nc = get_program(cfg, Tbc, boff, Tp, goff, layer)
        res = bass_utils.run_bass_kernel_spmd(
            nc, layer_maps[layer], core_ids=cores,
            trace=trace, trace_cores=trace_cores)
        layer_res.append(res)
        if res.exec_time_ns:
            total_ns += res.exec_time_ns
        if layer < 2:
            for r in range(cfg.NCORES):
                layer_maps[layer + 1][r]["x_in"] = res.results[r]["y_out"]
    out = assemble_output(cfg, layer_res[2].results, newlocal)
    return out, (total_ns, layer_res)


def kernel(**inputs) -> np.ndarray:
    return run(inputs)[0]



# revision 20
# speedup vs baseline: 4.0740x; 4.0740x over previous
"""3-layer GAT (graph attention network) forward pass on 8 Trainium2 cores.

v4 — balanced-coloring slot layout + fp8 layer-2 rows + PE tile-reduction:

  - nodes are 4-colored (color = 2-core chunk) by a greedy + local-search
    balancing pass so every dst node's in-edges split ~evenly across the 4
    chunks; within a color nodes sort by (max, lexicographic) chunk-degree
    so all four per-chunk block maxima align.  Padded-slot inflation drops
    from ~2.11x (per-core lexsort) to ~1.18x, which cuts the dominant cost
    (Q7 descriptor generation of the per-edge dma_gather) proportionally.
  - layer 2 table rows are 256B [W2h fp8e4 x160 | el f32 x4 | pad] instead
    of 512B bf16 rows: halves L2 gather+AllGather HBM traffic.
  - the tile-axis segment-sum no longer runs on DVE (tensor_reduce is
    capped at 1x): each edge-tile [128, F+H] of (alpha-weighted values |
    ee) is accumulated on the PE against an identity, giving acc and the
    softmax denominators in one PSUM tile.  Layers 0/1 use V as lhsT so
    the accumulated result lands feature-major (kills the extra PE
    transpose); layer 2 keeps slot-major via identity lhsT.
  - LeakyReLU is one fused scalar_tensor_tensor (max(x, 0.2x)); exp runs
    on ACT and writes straight into the V_ext tile's trailing ee columns.
  - bf16 on the PE; f32 attention logits; fp8 only for layer-2 values.
"""

import numpy as np
import ml_dtypes

import concourse.bass as bass
import concourse.bacc as bacc
import concourse.tile as tile
from concourse import mybir, bass_utils, library_config

F32 = mybir.dt.float32
I16 = mybir.dt.int16
BF16 = mybir.dt.bfloat16
F8 = mybir.dt.float8e4

BF_NP = ml_dtypes.bfloat16

NCHUNK = 4          # int16 gather chunks (2 cores each)


class Cfg:
    def __init__(self, n, e, ncores, in_dim=128, hid=16, heads=4, outc=40,
                 neg=0.2, eps=1e-5):
        assert n % ncores == 0
        self.N, self.E, self.NCORES = n, e, ncores
        self.IN, self.HID, self.HEADS, self.OUTC = in_dim, hid, heads, outc
        self.F = heads * hid          # 64
        self.F2 = heads * outc        # 160
        self.P = 128
        self.NPC = n // ncores        # nominal nodes per core
        self.BLOCKS = (self.NPC + self.P - 1) // self.P
        self.NOWN = self.BLOCKS * self.P   # padded nodes per core
        self.NPAD = ncores * self.NOWN
        self.NEG, self.EPS = neg, eps


CFG = Cfg(100000, 1600000, 8)


# ---------------------------------------------------------------- host prep

def _balanced_coloring(cfg, src, dst):
    """Assign each node a color in [0,4) (color c lives on cores 2c,2c+1)
    such that every dst's in-edges split ~evenly across colors.  Greedy
    init + a few passes of local search on sum_d max_c cnt[d,c]."""
    N = cfg.N
    out_deg = np.bincount(src, minlength=N)
    order_e = np.argsort(src, kind="stable")
    dst_by_src = dst[order_e]
    indptr = np.concatenate([[0], np.cumsum(out_deg)])
    CAP = 2 * cfg.NPC + 24         # slack for local search; rebalanced below
    proc = np.argsort(-out_deg, kind="stable")

    color = np.full(N, -1, np.int64)
    cnt = np.zeros((N, NCHUNK), np.int32)
    sizes = np.zeros(NCHUNK, np.int64)
    for s in proc:
        ds = dst_by_src[indptr[s]:indptr[s + 1]]
        sc = cnt[ds].sum(axis=0).astype(np.float64) if len(ds) else \
            np.zeros(NCHUNK)
        sc += sizes * 1e-5
        sc[sizes >= CAP] = np.inf
        c = int(np.argmin(sc))
        color[s] = c
        if len(ds):
            cnt[ds, c] += 1
        sizes[c] += 1

    for _ in range(8):
        moved = 0
        for s in proc:
            ds = dst_by_src[indptr[s]:indptr[s + 1]]
            if len(ds) == 0:
                continue
            c0 = color[s]
            cs = cnt[ds].astype(np.int64)
            eps = 1e-3
            cost0 = cs.max(axis=1).sum() + eps * (cs * cs).sum()
            best_c, best_d = c0, 0.0
            for c1 in range(NCHUNK):
                if c1 == c0 or sizes[c1] >= CAP:
                    continue
                t = cs.copy()
                t[:, c0] -= 1
                t[:, c1] += 1
                d = t.max(axis=1).sum() + eps * (t * t).sum() - cost0
                if d < best_d - 1e-12:
                    best_d, best_c = d, c1
            if best_c != c0:
                color[s] = best_c
                cnt[ds, c0] -= 1
                cnt[ds, best_c] += 1
                sizes[c0] -= 1
                sizes[best_c] += 1
                moved += 1
        if moved < 250:
            break

    # exact rebalance to 2*NPC per color so every core holds exactly NPC
    # real nodes (the yT pad memset and el-row patch rely on it).
    TGT = 2 * cfg.NPC
    while (sizes > TGT).any():
        c0 = int(np.argmax(sizes))
        under = np.where(sizes < TGT)[0]
        cand = np.where(color == c0)[0]
        best = (None, None, np.inf)
        for s in cand[np.argsort(out_deg[cand])[:256]]:
            ds = dst_by_src[indptr[s]:indptr[s + 1]]
            cs = cnt[ds].astype(np.int64)
            cost0 = cs.max(axis=1).sum() if len(ds) else 0
            for c1 in under:
                if len(ds):
                    t = cs.copy()
                    t[:, c0] -= 1
                    t[:, c1] += 1
                    d = t.max(axis=1).sum() - cost0
                else:
                    d = 0
                if d < best[2]:
                    best = (s, c1, d)
        s, c1, _ = best
        ds = dst_by_src[indptr[s]:indptr[s + 1]]
        color[s] = c1
        if len(ds):
            cnt[ds, c0] -= 1
            cnt[ds, c1] += 1
        sizes[c0] -= 1
        sizes[c1] += 1
    return color, cnt


def preprocess(cfg, src, dst):
    """Balanced-color binning + slot-per-partition chunked edge layout.

    Returns (idx16, Tbc, boff, Tp, goff, node_core, node_local)."""
    src = np.asarray(src, np.int64)
    dst = np.asarray(dst, np.int64)
    color, cnt = _balanced_coloring(cfg, src, dst)

    # within each color sort by (max, lex) of the 4-vector; deal rank k to
    # block k//256, core 2c+(k%2... of the within-block index), partition.
    node_core = np.empty(cfg.N, np.int64)
    node_local = np.empty(cfg.N, np.int64)
    cd = cnt.astype(np.int64)
    for c in range(NCHUNK):
        nodes = np.where(color == c)[0]
        v = cd[nodes]
        order = np.lexsort((-v[:, 3], -v[:, 2], -v[:, 1], -v[:, 0],
                            -v.max(axis=1)))
        nodes = nodes[order]
        k = np.arange(len(nodes))
        b = k // (2 * cfg.P)
        j = k % (2 * cfg.P)
        node_core[nodes] = 2 * c + (j & 1)
        node_local[nodes] = b * cfg.P + (j >> 1)

    newglobal = node_core * cfg.NOWN + node_local
    chunk_rows = 2 * cfg.NOWN
    chunk_of = node_core // 2
    local16 = (newglobal - chunk_of * chunk_rows).astype(np.int16)

    dl = node_local[dst]
    r_e = node_core[dst]
    b_e = dl // cfg.P
    p_e = dl % cfg.P
    c_e = chunk_of[src]

    # per (r, b, c, p) counts -> cross-core per (b, c) tile counts
    key = ((r_e * cfg.BLOCKS + b_e) * NCHUNK + c_e) * cfg.P + p_e
    cnt_e = np.bincount(key, minlength=cfg.NCORES * cfg.BLOCKS * NCHUNK * cfg.P)
    cnt4 = cnt_e.reshape(cfg.NCORES, cfg.BLOCKS, NCHUNK, cfg.P)
    Tbc = cnt4.max(axis=(0, 3)).astype(np.int64)          # [B, 4]
    boff = np.concatenate(
        [np.zeros((cfg.BLOCKS, 1), np.int64), np.cumsum(Tbc, axis=1)], axis=1)
    Tp = boff[:, NCHUNK]                                  # [B]
    goff = np.concatenate([[0], np.cumsum(Tp)]).astype(np.int64)
    TOT = int(goff[-1])

    order_e = np.argsort(key, kind="stable")
    gstart = np.concatenate([[0], np.cumsum(cnt_e)])
    rank = np.arange(cfg.E) - gstart[key[order_e]]

    bs = b_e[order_e]
    cs = c_e[order_e]
    tau = goff[bs] + boff[bs, cs] + rank
    flat = tau * cfg.P + p_e[order_e]
    rs = r_e[order_e]
    vals = local16[src][order_e]

    idx16 = np.full((cfg.NCORES, cfg.P * TOT), cfg.NOWN - 1, np.int16)
    idx16[rs, flat] = vals
    # wrap: flat position i lives at [i % 16, i // 16], replicated x8 rows
    wrapped = idx16.reshape(cfg.NCORES, cfg.P * TOT // 16, 16)
    wrapped = np.ascontiguousarray(wrapped.transpose(0, 2, 1))  # [NC,16,S]
    idx16 = np.ascontiguousarray(
        np.tile(wrapped, (1, 8, 1)))                      # [NC, 128, S]
    return idx16, Tbc.astype(int), boff.astype(int), Tp.astype(int), goff, \
        node_core, node_local


def _fold_el(W, a, heads, dph):
    return np.einsum("khd,hd->kh", W.reshape(W.shape[0], heads, dph),
                     a).astype(np.float32)


# ---------------------------------------------------------------- program

def build_layer_program(cfg, Tbc, boff, Tp, goff, layer):
    P, F, F2, IN = cfg.P, cfg.F, cfg.F2, cfg.IN
    H, D, OC = cfg.HEADS, cfg.HID, cfg.OUTC
    B = cfg.BLOCKS
    fo = F2 if layer == 2 else F
    fin = IN if layer == 0 else F
    # table rows are 256B for every layer now:
    #   layers 0/1: bf16 [Wh x64 | el f32 x4 | pad]     (ELEM=128 bf16 units)
    #   layer 2:    f32-typed [W2h fp8 x160 | el f32 x4 | pad] (ELEM=64 f32
    #   units; fp8 only via bitcast views so DMA/collectives stay f32)
    TD = F32 if layer == 2 else BF16
    ELEM = 64 if layer == 2 else 128
    ELOFF = 40 if layer == 2 else fo       # unit offset of el in row
    FOE = fo + H                           # V_ext width: values + ee cols
    Tpmax = int(max(Tp))
    TOT = int(goff[-1])
    GA = 7                              # phase-A write group (98 = 14*7)
    GI = 7                              # idx-load group
    assert B % GA == 0 and B % GI == 0
    islab_w = int(max(8 * (goff[g + GI] - goff[g])
                      for g in range(0, B, GI)))
    chunk_rows = 2 * cfg.NOWN

    nc = bacc.Bacc("TRN2", target_bir_lowering=False, debug=False,
                   num_devices=cfg.NCORES, num_swdge_queues=4)

    def inp(name, shape, dt=F32):
        return nc.dram_tensor(name, shape, dt, kind="ExternalInput").ap()

    x_in = inp("x_in", [fin, cfg.NOWN], BF16)
    idx_in = inp("idx16", [P, 8 * TOT], I16)
    Wext_i = inp("Wext", [fin, fo + 8], BF16)
    identb_i = inp("identb", [P, P], BF16)
    if layer < 2:
        rep4_i = inp("rep4", [H, F])
        gamma_i = inp("gamma", [F, 1])
        beta_i = inp("beta", [F, 1])
        y_out = nc.dram_tensor("y_out", [F, cfg.NOWN], BF16,
                               kind="ExternalOutput").ap()
    if layer == 0:
        resW_i = inp("resW", [IN, F], BF16)
    if layer == 2:
        resW_i = inp("resW", [F, F2], BF16)
        b2bc_in = inp("b2bc", [P, F2])
        out = nc.dram_tensor("out", [cfg.NOWN, F2], F32,
                             kind="ExternalOutput").ap()

    rg = [list(range(cfg.NCORES))]

    with tile.TileContext(nc) as tc:
        with (
            tc.tile_pool(name="big", bufs=1) as bigp,
            tc.tile_pool(name="const", bufs=1) as cons,
            tc.tile_pool(name="work", bufs=4) as wk,
            tc.tile_pool(name="idxp", bufs=3) as idxp,
            tc.tile_pool(name="grp", bufs=3) as grp,
            tc.tile_pool(name="ps", bufs=2, space="PSUM") as ps,
            tc.tile_pool(name="dram", bufs=1, space="DRAM") as dr,
        ):
            nc.gpsimd.load_library(library_config.mlp)

            tbl_own = dr.tile([cfg.NOWN, ELEM], TD)
            tbl_full = dr.tile([cfg.NPAD, ELEM], TD, addr_space="Shared")
            if layer < 2:
                stats_in = dr.tile([F, 2], F32)
                stats_out = dr.tile([F, 2], F32, addr_space="Shared")

            xT = bigp.tile([fin, cfg.NOWN], BF16, tag="xT")
            nc.sync.dma_start(out=xT[:], in_=x_in[:])
            ersb = bigp.tile([P, B * H], F32, tag="ersb")
            if layer < 2:
                yT = bigp.tile([F, cfg.NOWN], BF16, tag="yT")

            Wext = cons.tile([fin, fo + 8], BF16, tag="Wext")
            nc.sync.dma_start(out=Wext[:], in_=Wext_i[:])
            identb = cons.tile([P, P], BF16, tag="identb")
            nc.sync.dma_start(out=identb[:], in_=identb_i[:])
            if layer < 2:
                rep4 = cons.tile([H, F], F32, tag="rep4")
                nc.sync.dma_start(out=rep4[:], in_=rep4_i[:])
                gam = cons.tile([F, 1], F32, tag="gam")
                bet = cons.tile([F, 1], F32, tag="bet")
                nc.sync.dma_start(out=gam[:], in_=gamma_i[:])
                nc.sync.dma_start(out=bet[:], in_=beta_i[:])
                stats = cons.tile([F, 2], F32, tag="stats")
                nc.vector.memset(stats[:], 0.0)
            if layer == 0:
                rW = cons.tile([IN, F], BF16, tag="rW")
                nc.sync.dma_start(out=rW[:], in_=resW_i[:])
            if layer == 2:
                rW = cons.tile([F, F2], BF16, tag="rW")
                nc.sync.dma_start(out=rW[:], in_=resW_i[:])
                b2bc = cons.tile([P, F2], F32, tag="b2bc")
                nc.sync.dma_start(out=b2bc[:], in_=b2bc_in[:])

            # ---- phase A: table rows [Wh | el | pad] + er, AllGather ----
            for g in range(B // GA):
                grow = grp.tile([P, GA * ELEM], TD, tag="grow")
                for k in range(GA):
                    b = g * GA + k
                    tp = ps.tile([P, fo + 8], F32, tag="tp")
                    nc.tensor.matmul(out=tp[:], lhsT=xT[:, b * P:(b + 1) * P],
                                     rhs=Wext[:], start=True, stop=True)
                    if layer == 2:
                        nc.vector.tensor_copy(
                            out=grow[:, k * ELEM:k * ELEM + 40].bitcast(F8),
                            in_=tp[:, :160])
                        nc.vector.tensor_copy(
                            out=grow[:, k * ELEM + 40:k * ELEM + 44],
                            in_=tp[:, fo:fo + 4])
                    else:
                        seg = grow[:, k * ELEM:k * ELEM + fo + 8]
                        nc.vector.tensor_copy(out=seg[:, :fo],
                                              in_=tp[:, :fo])
                        nc.vector.tensor_copy(
                            out=seg[:, fo:fo + 8].bitcast(F32),
                            in_=tp[:, fo:fo + 4])
                    nc.vector.tensor_copy(out=ersb[:, b * H:(b + 1) * H],
                                          in_=tp[:, fo + 4:fo + 8])
                if g == B // GA - 1:
                    # pad-row patch: el of rows NOWN-32..NOWN (all zero pad
                    # nodes; every chunk's PADLOC row is one of these on an
                    # even core) = -1e38 so padded edge slots get ee = 0.
                    if layer == 2:
                        nc.vector.memset(
                            grow[P - 32:P, (GA - 1) * ELEM + 40:
                                 (GA - 1) * ELEM + 44], -1e38)
                    else:
                        nc.vector.memset(
                            grow[P - 32:P,
                                 (GA - 1) * ELEM + fo:(GA - 1) * ELEM + fo + 8]
                            .bitcast(F32), -1e38)
                nc.sync.dma_start(
                    out=tbl_own[g * GA * P:(g + 1) * GA * P, :]
                    .rearrange("(k p) f -> p k f", p=P),
                    in_=grow[:].rearrange("p (k f) -> p k f", k=GA))
            nc.gpsimd.collective_compute(
                "AllGather", mybir.AluOpType.bypass, replica_groups=rg,
                ins=[tbl_own[:].opt()], outs=[tbl_full[:].opt()])
            # gathers race the collective's remote writes without this:
            tc.strict_bb_all_engine_barrier()

            # ---- phase B: edge aggregation per block ----
            qrot = [0]
            for b in range(B):
                T = int(Tp[b])
                if b % GI == 0:
                    # idx slab for blocks [b, b+GI): wrapped columns
                    i0, i1 = 8 * int(goff[b]), 8 * int(goff[b + GI])
                    islab = idxp.tile([P, islab_w], I16, tag="islab")
                    nc.sync.dma_start(out=islab[:, :i1 - i0],
                                      in_=idx_in[:, i0:i1])
                    islab0 = i0
                G = wk.tile([P, Tpmax * ELEM], TD, tag="G")
                for c in range(NCHUNK):
                    Tc = int(Tbc[b, c])
                    bo = int(boff[b, c])
                    for s in range(0, Tc, 8):
                        w = min(8, Tc - s)
                        f0 = 8 * (int(goff[b]) + bo + s) - islab0
                        nc.gpsimd.dma_gather(
                            G[:, (bo + s) * ELEM:(bo + s + w) * ELEM]
                            .rearrange("p (j e) -> p j e", e=ELEM),
                            tbl_full[c * chunk_rows:(c + 1) * chunk_rows, :],
                            islab[:, f0:f0 + 8 * w],
                            128 * w, 128 * w, ELEM,
                            queue_num=qrot[0] % 4, single_packet=True)
                        qrot[0] += 1

                G3 = G[:, :T * ELEM].rearrange("p (t f) -> p t f", t=T)
                if layer == 2:
                    el = G3[:, :, ELOFF:ELOFF + H]         # [P, T, H] f32
                    gvals = G3[:, :, :ELOFF].bitcast(F8)   # [P, T, 160] fp8
                else:
                    el = G3[:, :, ELOFF:ELOFF + 8].bitcast(F32)
                    gvals = G3[:, :, :fo]

                ee = wk.tile([P, Tpmax * H], F32, tag="ee")
                ee3 = ee[:, :T * H].rearrange("p (t h) -> p t h", t=T)
                nc.vector.tensor_tensor(
                    out=ee3, in0=el,
                    in1=ersb[:, b * H:(b + 1) * H].unsqueeze(1)
                    .to_broadcast([P, T, H]),
                    op=mybir.AluOpType.add)
                # LeakyReLU: esc = max(NEG*x, x) in one fused DVE op
                esc = wk.tile([P, Tpmax * H], F32, tag="esc")
                nc.vector.scalar_tensor_tensor(
                    out=esc[:, :T * H], in0=ee[:, :T * H], scalar=cfg.NEG,
                    in1=ee[:, :T * H], op0=mybir.AluOpType.mult,
                    op1=mybir.AluOpType.max)

                # V_ext tile: [values * ee | ee] per edge slot
                V = wk.tile([P, Tpmax * FOE], BF16, tag="V")
                V3 = V[:, :T * FOE].rearrange("p (t f) -> p t f", t=T)
                nc.scalar.activation(
                    out=V3[:, :, fo:fo + H],
                    in_=esc[:, :T * H].rearrange("p (t h) -> p t h", t=T),
                    func=mybir.ActivationFunctionType.Exp)
                nc.vector.tensor_tensor(
                    out=V3[:, :, :fo].rearrange("p t (h d) -> p t h d", h=H),
                    in0=gvals.rearrange("p t (h d) -> p t h d", h=H),
                    in1=V3[:, :, fo:fo + H].unsqueeze(3).to_broadcast(
                        [P, T, H, OC if layer == 2 else D]),
                    op=mybir.AluOpType.mult)

                if layer < 2:
                    # PE tile-reduction, feature-major: psum [FOE, P] +=
                    # V_tile^T @ I for each edge tile.
                    acc = ps.tile([FOE, P], F32, tag="acc")
                    for t in range(T):
                        nc.tensor.matmul(out=acc[:], lhsT=V3[:, t, :],
                                         rhs=identb[:],
                                         start=(t == 0), stop=(t == T - 1))
                    dmax = wk.tile([H, P], F32, tag="dmax")
                    nc.vector.tensor_scalar_max(out=dmax[:],
                                                in0=acc[fo:fo + H, :],
                                                scalar1=1e-16)
                    rec = wk.tile([H, P], F32, tag="rec")
                    nc.vector.reciprocal(out=rec[:], in_=dmax[:])
                    # broadcast rec across each head's 16 feature rows
                    recb = ps.tile([F, P], F32, tag="recb")
                    nc.tensor.matmul(out=recb[:], lhsT=rep4[:], rhs=rec[:],
                                     start=True, stop=True)
                    recs = wk.tile([F, P], F32, tag="recs")
                    nc.vector.tensor_copy(out=recs[:], in_=recb[:])
                    rstn = wk.tile([F, P], F32, tag="rstn")
                    nc.vector.tensor_tensor(out=rstn[:], in0=acc[:fo, :],
                                            in1=recs[:],
                                            op=mybir.AluOpType.mult)

                    hsl = yT[:, b * P:(b + 1) * P]
                    if layer == 0:
                        rps = ps.tile([F, P], F32, tag="res")
                        nc.tensor.matmul(out=rps[:], lhsT=rW[:],
                                         rhs=xT[:, b * P:(b + 1) * P],
                                         start=True, stop=True)
                        nc.vector.tensor_tensor(out=hsl, in0=rstn[:],
                                                in1=rps[:],
                                                op=mybir.AluOpType.add)
                    else:
                        hold = xT[:, b * P:(b + 1) * P]
                        nc.vector.tensor_tensor(out=hsl, in0=rstn[:],
                                                in1=hold,
                                                op=mybir.AluOpType.add)
                        nc.vector.tensor_tensor(out=hsl, in0=hsl, in1=hold,
                                                op=mybir.AluOpType.add)

                    red = wk.tile([F, 1], F32, tag="red")
                    nc.vector.reduce_sum(out=red[:], in_=hsl,
                                         axis=mybir.AxisListType.X)
                    nc.vector.tensor_tensor(out=stats[:, 0:1],
                                            in0=stats[:, 0:1], in1=red[:],
                                            op=mybir.AluOpType.add)
                    sq = wk.tile([F, P], F32, tag="sq")
                    red2 = wk.tile([F, 1], F32, tag="red2")
                    nc.scalar.activation(
                        out=sq[:], in_=hsl,
                        func=mybir.ActivationFunctionType.Square,
                        accum_out=red2[:])
                    nc.vector.tensor_tensor(out=stats[:, 1:2],
                                            in0=stats[:, 1:2], in1=red2[:],
                                            op=mybir.AluOpType.add)
                else:
                    # PE tile-reduction, slot-major: psum [P, FOE] += I @ V
                    acc = ps.tile([P, FOE], F32, tag="acc")
                    for t in range(T):
                        nc.tensor.matmul(out=acc[:], lhsT=identb[:],
                                         rhs=V3[:, t, :],
                                         start=(t == 0), stop=(t == T - 1))
                    dmax = wk.tile([P, H], F32, tag="dmax")
                    nc.vector.tensor_scalar_max(out=dmax[:],
                                                in0=acc[:, fo:fo + H],
                                                scalar1=1e-16)
                    rec = wk.tile([P, H], F32, tag="rec")
                    nc.vector.reciprocal(out=rec[:], in_=dmax[:])
                    rps = ps.tile([P, F2], F32, tag="res")
                    nc.tensor.matmul(out=rps[:],
                                     lhsT=xT[:, b * P:(b + 1) * P],
                                     rhs=rW[:], start=True, stop=True)
                    rst = wk.tile([P, F2], F32, tag="rst")
                    nc.vector.tensor_tensor(
                        out=rst[:].rearrange("p (h o) -> p h o", h=H),
                        in0=acc[:, :fo].rearrange("p (h o) -> p h o", h=H),
                        in1=rec[:].unsqueeze(2).to_broadcast([P, H, OC]),
                        op=mybir.AluOpType.mult)
                    nc.vector.tensor_tensor(out=rst[:], in0=rst[:],
                                            in1=rps[:],
                                            op=mybir.AluOpType.add)
                    nc.vector.tensor_tensor(out=rst[:], in0=rst[:],
                                            in1=b2bc[:],
                                            op=mybir.AluOpType.add)
                    nc.sync.dma_start(out=out[b * P:(b + 1) * P, :],
                                      in_=rst[:])

            # ---- phase C: BN (layers 0/1) + writeback ----
            if layer < 2:
                nc.sync.dma_start(out=stats_in[:], in_=stats[:])
                nc.gpsimd.collective_compute(
                    "AllReduce", mybir.AluOpType.add, replica_groups=rg,
                    ins=[stats_in[:].opt()], outs=[stats_out[:].opt()])
                tc.strict_bb_all_engine_barrier()
                ssb = cons.tile([F, 2], F32, tag="ssb")
                nc.sync.dma_start(out=ssb[:], in_=stats_out[:])
                mu = cons.tile([F, 1], F32, tag="mu")
                tmp1 = cons.tile([F, 1], F32, tag="tmp1")
                scl = cons.tile([F, 1], F32, tag="scl")
                bia = cons.tile([F, 1], F32, tag="bia")
                musq = cons.tile([F, 1], F32, tag="musq")
                invn = 1.0 / cfg.N
                nc.vector.tensor_scalar_mul(out=mu[:], in0=ssb[:, 0:1],
                                            scalar1=invn)
                nc.vector.tensor_scalar_mul(out=tmp1[:], in0=ssb[:, 1:2],
                                            scalar1=invn)
                nc.scalar.activation(out=musq[:], in_=mu[:],
                                     func=mybir.ActivationFunctionType.Square)
                nc.vector.tensor_tensor(out=tmp1[:], in0=tmp1[:],
                                        in1=musq[:],
                                        op=mybir.AluOpType.subtract)
                nc.vector.tensor_scalar_add(out=tmp1[:], in0=tmp1[:],
                                            scalar1=cfg.EPS)
                nc.scalar.activation(out=tmp1[:], in_=tmp1[:],
                                     func=mybir.ActivationFunctionType.Sqrt)
                nc.vector.reciprocal(out=tmp1[:], in_=tmp1[:])
                nc.vector.tensor_tensor(out=scl[:], in0=tmp1[:], in1=gam[:],
                                        op=mybir.AluOpType.mult)
                nc.vector.tensor_tensor(out=tmp1[:], in0=mu[:], in1=scl[:],
                                        op=mybir.AluOpType.mult)
                nc.vector.tensor_tensor(out=bia[:], in0=bet[:], in1=tmp1[:],
                                        op=mybir.AluOpType.subtract)
                nc.scalar.activation(out=yT[:, :], in_=yT[:, :],
                                     func=mybir.ActivationFunctionType.Relu,
                                     scale=scl[:, 0:1], bias=bia[:, 0:1])
                if cfg.NPC < cfg.NOWN:
                    nc.vector.memset(yT[:, cfg.NPC:cfg.NOWN], 0.0)
                nc.sync.dma_start(out=y_out[:], in_=yT[:])

    nc.compile()
    return nc


# ---------------------------------------------------------------- host glue

def make_in_maps(cfg, inputs, idx16, node_core, node_local):
    feat = np.asarray(inputs["feat"], np.float32)
    H, D, OC = cfg.HEADS, cfg.HID, cfg.OUTC
    W0 = np.asarray(inputs["W0"], np.float32)
    W1 = np.asarray(inputs["W1"], np.float32)
    W2 = np.asarray(inputs["W2"], np.float32)
    W0ext = np.concatenate(
        [W0, _fold_el(W0, np.asarray(inputs["al0"]), H, D),
         _fold_el(W0, np.asarray(inputs["ar0"]), H, D)], axis=1).astype(BF_NP)
    W1ext = np.concatenate(
        [W1, _fold_el(W1, np.asarray(inputs["al1"]), H, D),
         _fold_el(W1, np.asarray(inputs["ar1"]), H, D)], axis=1).astype(BF_NP)
    W2ext = np.concatenate(
        [W2, _fold_el(W2, np.asarray(inputs["al2"]), H, OC),
         _fold_el(W2, np.asarray(inputs["ar2"]), H, OC)], axis=1).astype(BF_NP)

    identb = np.eye(cfg.P, dtype=np.float32).astype(BF_NP)
    # rep4[h, f] = 1 where f // HID == h: broadcasts per-head scalars to
    # each head's feature rows via a tiny PE matmul.
    rep4 = np.zeros((H, cfg.F), np.float32)
    for h in range(H):
        rep4[h, h * D:(h + 1) * D] = 1.0

    layer_maps = [[], [], []]
    for r in range(cfg.NCORES):
        sel = np.where(node_core == r)[0]
        fp = np.zeros((cfg.NOWN, cfg.IN), np.float32)
        fp[node_local[sel]] = feat[sel]
        common = {"idx16": idx16[r], "identb": identb}
        layer_maps[0].append({
            "x_in": np.ascontiguousarray(fp.T).astype(BF_NP),
            "Wext": W0ext,
            "rep4": rep4,
            "resW": np.asarray(inputs["resW0"], np.float32).astype(BF_NP),
            "gamma": np.asarray(inputs["gamma0"], np.float32).reshape(cfg.F, 1),
            "beta": np.asarray(inputs["beta0"], np.float32).reshape(cfg.F, 1),
            **common,
        })
        layer_maps[1].append({
            "Wext": W1ext,
            "rep4": rep4,
            "gamma": np.asarray(inputs["gamma1"], np.float32).reshape(cfg.F, 1),
            "beta": np.asarray(inputs["beta1"], np.float32).reshape(cfg.F, 1),
            **common,
        })
        layer_maps[2].append({
            "Wext": W2ext,
            "resW": np.asarray(inputs["resW2"], np.float32).astype(BF_NP),
            "b2bc": np.tile(np.asarray(inputs["b2"], np.float32)
                            .reshape(1, cfg.F2), (cfg.P, 1)),
            **common,
        })
    return layer_maps


def assemble_output(cfg, results, node_core, node_local):
    out = np.empty((cfg.N, cfg.F2), np.float32)
    for r in range(cfg.NCORES):
        sel = np.where(node_core == r)[0]
        out[sel] = results[r]["out"][node_local[sel]]
    return out


_PROG_CACHE = {}


def get_program(cfg, Tbc, boff, Tp, goff, layer):
    key = (cfg.N, cfg.E, int(goff[-1]), tuple(Tp), layer)
    if key not in _PROG_CACHE:
        _PROG_CACHE[key] = build_layer_program(cfg, Tbc, boff, Tp, goff, layer)
    return _PROG_CACHE[key]


def run(inputs, trace=False, trace_cores=None):
    cfg = CFG
    src = np.asarray(inputs["src"])
    dst = np.asarray(inputs["dst"])
    idx16, Tbc, boff, Tp, goff, node_core, node_local = \
        preprocess(cfg, src, dst)
    layer_maps = make_in_maps(cfg, inputs, idx16, node_core, node_local)
    cores = list(range(cfg.NCORES))
    total_ns = 0
    layer_res = []
    for layer in range(3):
        nc = get_program(cfg, Tbc, boff, Tp, goff, layer)
        res = bass_utils.run_bass_kernel_spmd(
            nc, layer_maps[layer], core_ids=cores,
            trace=trace, trace_cores=trace_cores)
        layer_res.append(res)
        if res.exec_time_ns:
            total_ns += res.exec_time_ns
        if layer < 2:
            for r in range(cfg.NCORES):
                layer_maps[layer + 1][r]["x_in"] = res.results[r]["y_out"]
    out = assemble_output(cfg, layer_res[2].results, node_core, node_local)
    return out, (total_ns, layer_res)


def kernel(**inputs) -> np.ndarray:
    return run(inputs)[0]
